# revision 8
# baseline (speedup 1.0000x reference)
"""Trainium2 Bass kernel for nn_AtenMmQuint8: quint8 dense matmul.

    out = ((x - 65) * 0.199) @ ((y - 160) * 0.0215)
    x: [2048, 4096] int32 (quint8 values 0..255)
    y: [4096, 2048] int32 (quint8 values 0..255)
    out: [2048, 2048] fp32

Sharding: 4x2 tensor-parallel grid over the 8 NeuronCores (4 M-blocks x
2 N-blocks); per-core DMA traffic is minimized at this grid shape and
each core's matmul work is identical (256 PE matmuls -> ~55us PE floor).

Host staging: the zero-point subtraction is done on the host for free:
(x - 65) in [-65, 190] and (y - 160) in [-160, 95] are integers, exactly
representable in bf16 (integers up to 256 are exact), so we ship bf16
operands and the device does NO dequant casts at all. This doubles DMA
bytes (12MB/core load, ~223 GB/s average demand vs ~358 GB/s available)
but frees the Vector engine entirely during the matmul stream and
removes every cast-wait stall from the PE critical path. x is staged
K-major (transposed) so the PE's stationary operand loads directly.

Device kernel (identical SPMD program on all 8 cores):
  - K is interleaved across SBUF partitions (k = p*kt + j) so each
    load-chunk DMA is 128 large contiguous runs (one per partition);
    the contraction is a permutation of K applied identically to both
    operands, so the matmul result is unchanged.
  - Progressively-sized load chunks (x on the SP HWDGE ring, y on the
    ACT ring, in parallel) into persistent bf16 SBUF buffers.
  - PE prewarm: throwaway matmuls while the first loads are in flight
    release the HAM clock gate (1.2 -> 2.4 GHz) just as the real
    stream starts.
  - PE matmul bf16 x bf16 -> fp32 at the ~215ns/matmul roofline,
    accumulating the whole 512x1024 block across all 8 PSUM banks
    k-outer (PE never waits on a full K pass); the last 8 k-tiles run
    (m, n)-major so banks retire one at a time and their copy+store
    overlaps the remaining matmuls, keeping the kernel-ending chain
    short (one 512-col copy + one 256KB store).
  - Scale+copy PSUM -> SBUF fused with the combined scale on VectorE
    (otherwise idle), one store DMA per retired PSUM bank.
"""

import numpy as np

import concourse.bass as bass  # noqa: F401  (kept for callers/debugging)
import concourse.mybir as mybir
import concourse.tile as tile
from concourse import bacc
from concourse.bass_utils import run_bass_kernel_spmd

X_ZP, Y_ZP = 65.0, 160.0
SCALE = 0.199 * 0.0215

M, K, N = 2048, 4096, 2048
GM, GN = 4, 2  # core grid: 4 M-blocks x 2 N-blocks
MC, NC = M // GM, N // GN  # 512 x 1024 per-core output block
P = 128  # partitions / k-tile size
NB = 512  # psum bank free size (one fp32 bank; matmul cannot cross banks)
# k-tiles per load DMA chunk: small leading chunks start the pipeline
# early (the PE only ever waits on a 1-2 k-tile transfer), moderate
# trailing chunks amortize per-DMA completion overhead while keeping
# the wait granularity fine enough that one chunk's completion latency
# (~1-2us receipt) never outruns the PE's 1.72us/k-tile consumption.
DMA_CHUNKS = (1, 1, 1, 1, 2, 2, 2, 2, 2, 2, 2, 2, 4, 4, 4)
KT_TAIL = 8  # trailing k-tiles run (m,n)-major so PSUM banks retire early
N_WARM = 34


def _emit(tc, xT, ys, out, dma_chunks=DMA_CHUNKS, kt_tail=KT_TAIL, n_warm=N_WARM):
    """Emit the per-core device program.

    xT: [k, mc] bf16 DRAM (x slice, K-major, zero-point subtracted),
    ys: [k, nnc] bf16 DRAM (zero-point subtracted),
    out: [mc, nnc] fp32 DRAM.
    """
    nc = tc.nc
    k, mc = xT.shape
    nnc = ys.shape[1]
    kt = k // P
    mt = mc // P
    nt = nnc // NB
    assert sum(dma_chunks) == kt

    fp32 = mybir.dt.float32
    bf16 = mybir.dt.bfloat16

    with (
        tc.tile_pool(name="sb", bufs=1) as sbp,
        tc.tile_pool(name="osb", bufs=mt * nt, space="SBUF") as osbp,
        tc.tile_pool(name="ps", bufs=mt * nt, space="PSUM") as psp,
    ):
        # Everything is persistent (fits in SBUF at this problem size):
        # each DMA writes a disjoint slice, so instructions don't accrue
        # buffer-recycling waits.
        xb = sbp.tile([P, kt, mc], bf16, name="xb")
        yb = sbp.tile([P, kt, nnc], bf16, name="yb")
        wt = sbp.tile([P, P], bf16, name="wt")
        psum = [
            [psp.tile([P, NB], fp32, tag="ps", name=f"ps_{m}_{n}") for n in range(nt)]
            for m in range(mt)
        ]

        # K interleaved across partitions (k = p*kt + j): each
        # partition's j-range is one contiguous DRAM run, so a chunk DMA
        # is 128 big descriptors instead of 128*nk small ones.
        xTr = xT.rearrange("(p j) m -> p j m", j=kt)
        ysr = ys.rearrange("(p j) n -> p j n", j=kt)
        k0 = 0
        for ci, nk in enumerate(dma_chunks):
            nc.sync.dma_start(xb[:, k0 : k0 + nk, :], xTr[:, k0 : k0 + nk, :])
            # y-loads issue from the ACT HWDGE ring, in parallel with the
            # x-load issues on the SP ring. The very first y tile is
            # split into column halves so the first matmuls (which read
            # only cols 0..NB) can start ~0.7us earlier.
            if ci == 0 and nk == 1:
                nc.scalar.dma_start(yb[:, k0, :NB], ysr[:, k0, :NB])
                nc.scalar.dma_start(yb[:, k0, NB:], ysr[:, k0, NB:])
            else:
                nc.scalar.dma_start(yb[:, k0 : k0 + nk, :], ysr[:, k0 : k0 + nk, :])
            k0 += nk

        # HAM prewarm: the PE sits idle for ~3 us while the first chunk
        # loads; throwaway matmuls release the clock gate to 8/8 before
        # the real stream starts. memset on VectorE (it is otherwise
        # idle until the PSUM copies at the end).
        nc.vector.memset(wt[:], 0.0)
        for _ in range(n_warm):
            nc.tensor.matmul(psum[0][0][:, :P], wt[:], wt[:], start=True, stop=True)

        def mm(j, m, n):
            nc.tensor.matmul(
                psum[m][n][:],
                xb[:, j, m * P : (m + 1) * P],
                yb[:, j, n * NB : (n + 1) * NB],
                start=(j == 0),
                stop=(j == kt - 1),
            )

        # k-outer: touch every psum bank each k-tile so the PE stream
        # stays dense while loads race ahead. The first k-tile runs
        # n-outer so its first 4 matmuls need only the first y column
        # half (loaded by the smaller leading DMA).
        for n in range(nt):
            for m in range(mt):
                mm(0, m, n)
        for j in range(1, kt - kt_tail):
            for m in range(mt):
                for n in range(nt):
                    mm(j, m, n)
        # (m, n)-major tail: each PSUM bank finishes its K accumulation
        # alone, so its scale-copy + store overlaps the remaining
        # matmuls of the other banks.
        for m in range(mt):
            for n in range(nt):
                last = m == mt - 1 and n == nt - 1
                for j in range(kt - kt_tail, kt):
                    mm(j, m, n)
                osb = osbp.tile([P, NB], fp32, tag="osb", name=f"osb_{m}_{n}")
                nc.vector.tensor_scalar_mul(osb[:], psum[m][n][:], SCALE)
                if last:
                    # split the kernel-ending store across both HWDGE
                    # rings so the two halves transfer in parallel
                    h = NB // 2
                    nc.sync.dma_start(
                        out[m * P : (m + 1) * P, n * NB : n * NB + h], osb[:, :h]
                    )
                    nc.scalar.dma_start(
                        out[m * P : (m + 1) * P, n * NB + h : (n + 1) * NB], osb[:, h:]
                    )
                else:
                    nc.sync.dma_start(
                        out[m * P : (m + 1) * P, n * NB : (n + 1) * NB], osb[:]
                    )


def _build_nc(k=K, mc=MC, nnc=NC, **emit_kw):
    nc = bacc.Bacc("TRN2", target_bir_lowering=False, debug=False)
    xT = nc.declare_dram_parameter("xT", [k, mc], mybir.dt.bfloat16, isOutput=False)
    ys = nc.declare_dram_parameter("ys", [k, nnc], mybir.dt.bfloat16, isOutput=False)
    out = nc.declare_dram_parameter("out", [mc, nnc], mybir.dt.float32, isOutput=True)
    with tile.TileContext(nc) as tc:
        _emit(tc, xT[:], ys[:], out[:], **emit_kw)
    nc.compile()
    return nc


_CACHE = {}


def _get_nc():
    if "nc" not in _CACHE:
        _CACHE["nc"] = _build_nc()
    return _CACHE["nc"]


def kernel(x, y):
    x = np.asarray(x)
    y = np.asarray(y)
    assert x.shape == (M, K) and y.shape == (K, N)
    bf16 = mybir.dt.np(mybir.dt.bfloat16)
    # Zero-point subtraction on the host: the results are integers in
    # [-160, 190], exactly representable in bf16, so the device needs no
    # dequant work at all. x is staged K-major for the PE's stationary
    # operand.
    xT_bf = (x.T.astype(np.float32) - X_ZP).astype(bf16)
    y_bf = (y.astype(np.float32) - Y_ZP).astype(bf16)

    in_maps = []
    for i in range(GM * GN):
        mi, ni = divmod(i, GN)
        in_maps.append(
            {
                "xT": np.ascontiguousarray(xT_bf[:, mi * MC : (mi + 1) * MC]),
                "ys": np.ascontiguousarray(y_bf[:, ni * NC : (ni + 1) * NC]),
            }
        )

    res = run_bass_kernel_spmd(_get_nc(), in_maps, list(range(GM * GN)))
    _CACHE["last_results"] = res

    out = np.empty((M, N), np.float32)
    for i in range(GM * GN):
        mi, ni = divmod(i, GN)
        out[mi * MC : (mi + 1) * MC, ni * NC : (ni + 1) * NC] = res.results[i]["out"]
    return out
